# revision 5
# baseline (speedup 1.0000x reference)
"""DeltaNet chunked delta-rule kernel for Trainium2 (Bass/Tile), 8-core SPMD.

Full inputs: q,k,v [4,8,4096,128] fp32, beta [4,8,4096] fp32.
Sharding: 32 (b,h) pairs -> 4 per core across 8 cores (state S is per (b,h)).

Algorithm (identical to the CHUNK=32 reference for any chunk size; C=128):
  kh = l2norm(k), per chunk: T = (beta*kh) @ kh^T; P = -stril(T)
  inv = (I+P)(I+P2)(I+P4)  (truncated Neumann, exact through P^7)
  u = inv @ diag(beta) @ (v - kh@S); out = l2norm(q)@S + tril(qh kh^T)@u
  S += kh^T u

v4 design notes (vs the 381us baseline, trace-driven):
- Per-instruction fixed costs dominate at [128,128] tile sizes (DVE ~190ns,
  ACT ~300ns, PE MM ~56-80ns warm). So: minimize op COUNT everywhere, put
  adds on the PE (identity matmuls), merge drains into multi-slot ops across
  chunk PAIRS (PSUM ring slots are address-adjacent), keep the PE stream
  dense so the HAM clock stays at 2.4GHz (the old kernel ran cold at 1.2).
- Host precomputes rk=rsqrt(|k|^2), beta*rk, beta (shipped as one small fp32
  tensor); rq is applied to the OUTPUT on the host (both out terms are
  linear in qh's rows), so q is used RAW on device and no q-norm, no qh, no
  vb materialization exist on device.
- Inputs packed+cast to bf16 on host as [nseq, NG, C, 3, G, D] so each
  group DMA is one descriptor of 3KB per partition (line rate); output
  written bf16 per group and unpacked+rq-scaled+fp32-cast on host.
- Chain/scan adds ride PSUM accumulation; S is accumulated across all 32
  chunks in a persistent PSUM bank (start=True only on the very first
  matmul of the bank per repeat); Sb is a per-iter 4-seq merged bf16 cast.
- invT is drained row-scaled by beta (invT' = diag(beta)@invT), paired
  across chunks with a stride-0-broadcast in1 multiply.
"""
import numpy as np
import ml_dtypes

import concourse.bass as bass
import concourse.mybir as mybir
import concourse.tile as tile
from concourse import bacc
from concourse.bass_utils import run_bass_kernel_spmd
from concourse.masks import make_identity, make_lower_triangular, make_upper_triangular

B, H, L, D = 4, 8, 4096, 128
C = 128
NT = L // C
G = 4                 # chunks per load-group
NG = NT // G
NSEQ = (B * H) // 8   # sequences per core
FP = mybir.dt.float32
BF = mybir.dt.bfloat16
EPS = 1e-6
AF = mybir.ActivationFunctionType
ALU = mybir.AluOpType
BF_NP = ml_dtypes.bfloat16


def _emit_pair_pre(nc, work, psum, cst, scal_t, qkv, s, j):
    """Pre-scan work for chunks (2j, 2j+1) of one sequence: kh/kb, transposes,
    T/Tt/attnT, Neumann chain -> invT' (beta-row-scaled), all pair-merged."""
    identB = cst["identB"]
    kh2 = work.tile([C, 2, D], BF, tag="kh2", name="kh2")
    kb2 = work.tile([C, 2, D], BF, tag="kb2", name="kb2")
    for jj in range(2):
        i = 2 * j + jj
        ci = i % G
        kg = qkv[:, 1, ci, :]
        nc.scalar.activation(out=kh2[:, jj, :], in_=kg, func=AF.Copy,
                             scale=scal_t[:, i, 0:1])
        nc.gpsimd.tensor_scalar(out=kb2[:, jj, :], in0=kg,
                                scalar1=scal_t[:, i, 1:2], scalar2=1.0,
                                op0=ALU.mult, op1=ALU.mult)

    # 6 transposes into one bf16 psum bank; slots per chunk: (kT, kbT, qT)
    tr_ps = psum.tile([D, 6, C], BF, tag="trp", name="tr_ps", bufs=1)
    for jj in range(2):
        i = 2 * j + jj
        ci = i % G
        nc.tensor.matmul(tr_ps[:, 3 * jj + 0, :], kh2[:, jj, :], identB, is_transpose=True)
        nc.tensor.matmul(tr_ps[:, 3 * jj + 1, :], kb2[:, jj, :], identB, is_transpose=True)
        nc.tensor.matmul(tr_ps[:, 3 * jj + 2, :], qkv[:, 0, ci, :], identB, is_transpose=True)
    trs = work.tile([D, 6, C], BF, tag="trs", name="trs")
    nc.vector.tensor_copy(trs, tr_ps)

    # ta: per chunk slot0 = T = kb@kh^T, slots1:3 = [Tt | attnT_raw] (one wide MM)
    ta_ps = psum.tile([C, 2, 3, C], FP, tag="tap", name="ta_ps", bufs=1)
    for jj in range(2):
        kT = trs[:, 3 * jj + 0, :]
        kbT = trs[:, 3 * jj + 1, :]
        nc.tensor.matmul(ta_ps[:, jj, 0, :], kbT, kT)
        nc.tensor.matmul(ta_ps[:, jj, 1:3, :], kT, trs[:, 3 * jj + 1:3 * jj + 3, :])
    ppa = work.tile([C, 2, 3, C], BF, tag="ppa", name="ppa")
    nc.vector.tensor_tensor(out=ppa, in0=ta_ps, in1=cst["mPA3"], op=ALU.mult)

    # chain bank: [C, 2(chunk), 2(slot), C], 3-stage slot reuse
    ch_ps = psum.tile([C, 2, 2, C], FP, tag="chp", name="ch_ps", bufs=2)
    # stage A: P2 = P1@P1 (via PT1 stationary), PT2 = (P1@P1)^T
    for jj in range(2):
        P1, PT1 = ppa[:, jj, 0, :], ppa[:, jj, 1, :]
        nc.tensor.matmul(ch_ps[:, jj, 0, :], PT1, P1)
        nc.tensor.matmul(ch_ps[:, jj, 1, :], P1, PT1)
    PPa = work.tile([C, 2, 2, C], BF, tag="PPa", name="PPa")
    nc.vector.tensor_copy(PPa, ch_ps)
    # stage B: slot0 = P4, slot1 = R1 = I + PT1 + PT2 + PT2@PT1
    for jj in range(2):
        PT1 = ppa[:, jj, 1, :]
        P2, PT2 = PPa[:, jj, 0, :], PPa[:, jj, 1, :]
        nc.tensor.matmul(ch_ps[:, jj, 0, :], PT2, P2)
        nc.tensor.matmul(ch_ps[:, jj, 1, :], identB, identB, start=True, stop=False)
        nc.tensor.matmul(ch_ps[:, jj, 1, :], identB, PT1, start=False, stop=False)
        nc.tensor.matmul(ch_ps[:, jj, 1, :], P2, identB, start=False, stop=False)
        nc.tensor.matmul(ch_ps[:, jj, 1, :], P2, PT1, start=False, stop=True)
    PPb = work.tile([C, 2, 2, C], BF, tag="PPb", name="PPb")
    nc.scalar.copy(PPb, ch_ps)
    # stage C: slot0 = V = (I + PT4) @ R1
    for jj in range(2):
        P4, R1 = PPb[:, jj, 0, :], PPb[:, jj, 1, :]
        nc.tensor.matmul(ch_ps[:, jj, 0, :], P4, R1, start=True, stop=False)
        nc.tensor.matmul(ch_ps[:, jj, 0, :], identB, R1, start=False, stop=True)
    invT2 = work.tile([C, 2, C], BF, tag="invT2", name="invT2")
    bpair = scal_t[:, 2 * j:2 * j + 2, 2:3].broadcast_to([C, 2, C])
    nc.vector.tensor_tensor(out=invT2,
                            in0=ch_ps[:, :, 0, :],
                            in1=bpair, op=ALU.mult)
    return dict(trs=trs, ppa=ppa, invT2=invT2, kh2=kh2)


def _emit_scan(nc, work, psum, dram, pre, S4, Sb4, qkv, outg, s, i, first):
    """Serial per-chunk scan: z, y, u, o, S update."""
    j, jj = i // 2, i % 2
    ci = i % G
    trs, ppa, invT2, kh2 = pre["trs"], pre["ppa"], pre["invT2"], pre["kh2"]
    kT = trs[:, 3 * jj + 0, :]
    qT = trs[:, 3 * jj + 2, :]
    attnT = ppa[:, jj, 2, :]
    sc = psum.tile([C, 2, D], FP, tag="scp", name="sc", bufs=2)
    nc.tensor.matmul(sc[:, 0, :], kT, Sb4[:, s, :])            # z = kh@Sb
    y = work.tile([C, D], BF, tag="y", name="y")
    nc.vector.tensor_tensor(out=y, in0=qkv[:, 2, ci, :], in1=sc[:, 0, :],
                            op=ALU.subtract)                   # y = v - z
    nc.tensor.matmul(sc[:, 1, :], invT2[:, jj, :], y)          # u = inv b y
    nc.tensor.matmul(sc[:, 0, :], qT, Sb4[:, s, :], start=True, stop=False)
    u_bf = work.tile([C, D], BF, tag="u_bf", name="u_bf")
    nc.scalar.copy(u_bf, sc[:, 1, :])
    nc.tensor.matmul(sc[:, 0, :], attnT, u_bf, start=False, stop=True)
    nc.scalar.copy(outg[:, ci, :], sc[:, 0, :])
    nc.tensor.matmul(S4[:, s, :], kh2[:, jj, :], u_bf,
                     start=first, stop=(i == NT - 1), skip_group_check=True)


def build_nc(nseq=NSEQ, nt=NT, repeat=1):
    assert nt % (2 * G) == 0
    ng = nt // G
    nc = bacc.Bacc(None, target_bir_lowering=False)
    dram = {
        "qkv": nc.dram_tensor("qkv", [nseq, ng, C, 3, G, D], BF, kind="ExternalInput"),
        "scal": nc.dram_tensor("scal", [nseq, C, nt, 3], FP, kind="ExternalInput"),
        "out": nc.dram_tensor("out", [nseq, ng, C, G, D], BF, kind="ExternalOutput"),
    }
    with tile.TileContext(nc) as tc:
        with (
            tc.tile_pool(name="consts", bufs=1) as consts,
            tc.tile_pool(name="persist", bufs=1) as persist,
            tc.tile_pool(name="grp", bufs=8) as grp,
            tc.tile_pool(name="work", bufs=6) as work,
            tc.tile_pool(name="psum", bufs=1, space="PSUM") as psum,
        ):
            identF = consts.tile([128, 128], FP, tag="identF", name="identF")
            identB = consts.tile([128, 128], BF, tag="identB", name="identB")
            mPA3 = consts.tile([C, 2, 3, C], FP, tag="mPA3", name="mPA3")
            make_identity(nc, identF)
            nc.vector.tensor_copy(identB, identF)
            for jj in range(2):
                make_lower_triangular(nc, mPA3[:, jj, 0, :], val=-1.0, diag=False)
                make_upper_triangular(nc, mPA3[:, jj, 1, :], val=-1.0, diag=False)
                make_upper_triangular(nc, mPA3[:, jj, 2, :], val=1.0, diag=True)
            cst = dict(identB=identB, mPA3=mPA3)

            scal_t = []
            for s in range(nseq):
                st = persist.tile([C, nt, 3], FP, tag=f"scal{s}", name=f"scal{s}")
                nc.sync.dma_start(out=st, in_=dram["scal"][s])
                scal_t.append(st)
            Sb4 = persist.tile([D, nseq, D], BF, tag="Sb4", name="Sb4")
            S4 = psum.tile([D, nseq, D], FP, tag="S4", name="S4", bufs=1)

            for rep in range(repeat):
                nc.gpsimd.memset(Sb4, 0.0)
                for g in range(ng):
                    qkv_t, outg_t = [], []
                    for s in range(nseq):
                        qt = grp.tile([C, 3, G, D], BF, tag="qkv", name="qkv_t")
                        nc.sync.dma_start(out=qt, in_=dram["qkv"][s, g])
                        qkv_t.append(qt)
                        outg_t.append(grp.tile([C, G, D], BF, tag="outg",
                                               name="outg"))
                    for j2 in range(G // 2):
                        j = g * (G // 2) + j2
                        pres = [_emit_pair_pre(nc, work, psum, cst, scal_t[s],
                                               qkv_t[s], s, j)
                                for s in range(nseq)]
                        for jj in range(2):
                            i = 2 * j + jj
                            for s in range(nseq):
                                _emit_scan(nc, work, psum, dram, pres[s], S4, Sb4,
                                           qkv_t[s], outg_t[s], s, i,
                                           first=(s == 0 and i == 0))
                            if i < nt - 1:
                                nc.vector.tensor_copy(Sb4, S4)
                    for s in range(nseq):
                        nc.sync.dma_start(out=dram["out"][s, g],
                                          in_=outg_t[s])
    nc.compile()
    return nc


_NC_CACHE = None


def _prep_inputs(q, k, v, beta):
    """Host-side: fp32 norms, bf16 pack [nseq, NG, C, 3, G, D], scal tensor."""
    nseq_all = B * H
    qf = np.ascontiguousarray(np.asarray(q, dtype=np.float32)).reshape(nseq_all, L, D)
    kf = np.ascontiguousarray(np.asarray(k, dtype=np.float32)).reshape(nseq_all, L, D)
    vf = np.ascontiguousarray(np.asarray(v, dtype=np.float32)).reshape(nseq_all, L, D)
    bf_ = np.ascontiguousarray(np.asarray(beta, dtype=np.float32)).reshape(nseq_all, L)
    rq = 1.0 / np.sqrt((qf.astype(np.float64) ** 2).sum(-1) + EPS)
    rk = 1.0 / np.sqrt((kf.astype(np.float64) ** 2).sum(-1) + EPS)
    rq = rq.astype(np.float32)
    rk = rk.astype(np.float32)
    # qkv packed: [nseq, NG, G, C, D] -> [nseq, NG, C, 3, G, D]
    def pack(x):
        return x.reshape(nseq_all, NG, G, C, D).transpose(0, 1, 3, 2, 4)
    qkv = np.stack([pack(qf), pack(kf), pack(vf)], axis=3)  # [ns, NG, C, 3, G, D]
    qkv = np.ascontiguousarray(qkv.astype(BF_NP))
    # scal: (rk, beta*rk, beta) -> [nseq, C, NT, 3]
    scal = np.stack([rk, bf_ * rk, bf_], axis=-1)           # [ns, L, 3]
    scal = scal.reshape(nseq_all, NT, C, 3).transpose(0, 2, 1, 3)
    scal = np.ascontiguousarray(scal.astype(np.float32))
    return qkv, scal, rq


def kernel(q, k, v, beta):
    global _NC_CACHE
    if _NC_CACHE is None:
        _NC_CACHE = build_nc()
    nc = _NC_CACHE
    qkv, scal, rq = _prep_inputs(q, k, v, beta)
    in_maps = []
    for core in range(8):
        sl = slice(core * NSEQ, (core + 1) * NSEQ)
        in_maps.append({
            "qkv": np.ascontiguousarray(qkv[sl]),
            "scal": np.ascontiguousarray(scal[sl]),
        })
    res = run_bass_kernel_spmd(nc, in_maps, core_ids=list(range(8)))
    out = np.empty((B * H, L, D), dtype=np.float32)
    for core in range(8):
        ob = np.asarray(res.results[core]["out"], dtype=np.float32)
        # [NSEQ, NG, C, G, D] -> [NSEQ, NG, G, C, D] -> [NSEQ, L, D]
        sl = slice(core * NSEQ, (core + 1) * NSEQ)
        out[sl] = ob.transpose(0, 1, 3, 2, 4).reshape(NSEQ, L, D)
    out *= rq[:, :, None]
    return out.reshape(B, H, L, D)
